# revision 16
# baseline (speedup 1.0000x reference)
"""Trainium2 Bass kernel for NibbleMulFFN.

Reference semantics: a, b are exact one-hot [N,16] fp32 rows. The network
computes address = softmax(100 * (concat(a,b) @ W1)) which is exactly
one-hot at index a_idx*16+b_idx in fp32 (winning logit 200, runners-up 100
-> exp gap e^-100 ~ 3.8e-44, denormal), then lo = address @ W2_lo and
hi = address @ W2_hi, i.e. one-hot of (a_idx*b_idx) & 15 and
(a_idx*b_idx) >> 4.

Kernel strategy (pure data parallel over 8 cores, batch split):
  per 128xF-row tile (batch rows packed F-per-partition, contiguous DMA):
    1. load a,b slices into one SBUF tile [128, 2*F*16]
    2. val extraction: multiply by broadcast iota (0..15) and
       segmented reduce-sum over the 16-lane groups -> a_idx|b_idx [128, 2F]
    3. v = a_idx*b_idx; lo = v mod 16; hi = (v - lo)/16  (all exact in fp32)
    4. one-hot: is_equal(val broadcast, iota broadcast) -> [128, 2F, 16]
    5. store lo/hi one-hot slices
The heavy elementwise multiply runs on GPSIMD so the Vector engine only
pays for the reduce + is_equal, keeping DVE below the ~45us/core memory
roofline (16 MB of HBM traffic per core at ~358 GB/s).
"""

import numpy as np

N_TOTAL = 524288
N_CORES = 8
N_CORE = N_TOTAL // N_CORES  # 65536 rows per core
P = 128  # SBUF partitions
K = 16  # one-hot width

# Tunables
F = 128  # batch rows per partition per tile -> tile covers P*F = 16384 rows
# NOTE: tensor_tensor is not a valid opcode on the Pool (GPSIMD) engine on
# TRN2 — all elementwise/reduce work must run on the Vector engine.
MULT_ENGINE = "vector"
EQ_HI_ENGINE = "vector"
WORK_BUFS = 3

_CACHE = {}


def _build_nc(n_rows, f=F, mult_engine=MULT_ENGINE, eq_hi_engine=EQ_HI_ENGINE,
              repeats=1, work_bufs=WORK_BUFS, variant="full", bf=256):
    import concourse.bacc as bacc
    import concourse.mybir as mybir
    from concourse.tile import TileContext

    fk = f * K
    assert n_rows % (P * f) == 0
    T = n_rows // (P * f)

    nc = bacc.Bacc("TRN2", target_bir_lowering=False, debug=False)
    dt = mybir.dt.float32
    a_in = nc.dram_tensor("a_in", [n_rows, K], dt, kind="ExternalInput").ap()
    b_in = nc.dram_tensor("b_in", [n_rows, K], dt, kind="ExternalInput").ap()
    iota_in = nc.dram_tensor("iota_in", [P, K], dt, kind="ExternalInput").ap()
    lo_out = nc.dram_tensor("lo_out", [n_rows, K], dt, kind="ExternalOutput").ap()
    hi_out = nc.dram_tensor("hi_out", [n_rows, K], dt, kind="ExternalOutput").ap()

    # [T, 128, F*K] views; tile t / partition p holds rows t*P*f + p*f ... +f
    a_t = a_in.rearrange("(t p f) k -> t p (f k)", p=P, f=f)
    b_t = b_in.rearrange("(t p f) k -> t p (f k)", p=P, f=f)
    lo_t = lo_out.rearrange("(t p f) k -> t p (f k)", p=P, f=f)
    hi_t = hi_out.rearrange("(t p f) k -> t p (f k)", p=P, f=f)

    mult = mybir.AluOpType.mult
    if variant == "bigload":
        return _build_nc_bigload(nc, mybir, n_rows, f, repeats, a_in, b_in,
                                 iota_in, lo_out, hi_out, bf=bf)
    with TileContext(nc) as tc:
        with (
            tc.tile_pool(name="const", bufs=1) as cpool,
            tc.tile_pool(name="work", bufs=work_bufs) as pool,
        ):
            iota = cpool.tile([P, K], dt)
            nc.sync.dma_start(out=iota[:], in_=iota_in)
            iota_i = cpool.tile([P, K], mybir.dt.int32)
            nc.gpsimd.iota(iota_i[:], pattern=[[1, K]], base=0, channel_multiplier=0)
            meng = nc.gpsimd if mult_engine == "gpsimd" else nc.vector
            eq_hi_eng = nc.gpsimd if eq_hi_engine == "gpsimd" else nc.vector
            for t in [ti for _ in range(repeats) for ti in range(T)]:
                ab = pool.tile([P, 2 * fk], dt, tag="ab")
                if variant == "split2":
                    h = fk // 2
                    nc.sync.dma_start(out=ab[:, :h], in_=a_t[t][:, :h])
                    nc.sync.dma_start(out=ab[:, h:fk], in_=a_t[t][:, h:])
                    nc.sync.dma_start(out=ab[:, fk : fk + h], in_=b_t[t][:, :h])
                    nc.sync.dma_start(out=ab[:, fk + h :], in_=b_t[t][:, h:])
                else:
                    nc.sync.dma_start(out=ab[:, :fk], in_=a_t[t])
                    nc.sync.dma_start(out=ab[:, fk:], in_=b_t[t])
                if variant == "dma_only":
                    nc.sync.dma_start(out=lo_t[t], in_=ab[:, :fk])
                    nc.sync.dma_start(out=hi_t[t], in_=ab[:, fk:])
                    continue
                ab3 = ab[:].rearrange("p (s k) -> p s k", k=K)  # [P, 2F, 16]
                iota_b = iota[:, None, :].broadcast_to([P, 2 * f, K])
                ij = pool.tile([P, 2 * f], dt, tag="ij")  # a_idx | b_idx
                if variant == "no_val":
                    # skip mult+reduce; fabricate ij from raw data
                    nc.vector.tensor_copy(out=ij[:], in_=ab[:, : 2 * f])
                else:
                    # ab *= iota  (in place)
                    meng.tensor_tensor(out=ab3, in0=ab3, in1=iota_b, op=mult)
                    nc.vector.tensor_reduce(
                        out=ij[:], in_=ab3, axis=mybir.AxisListType.X,
                        op=mybir.AluOpType.add,
                    )
                if variant == "no_eq":
                    nc.sync.dma_start(out=lo_t[t], in_=ab[:, :fk])
                    nc.sync.dma_start(out=hi_t[t], in_=ab[:, fk:])
                    continue
                # v = a_idx * b_idx, then to int32 (exact: v is an integer <= 225)
                vf = pool.tile([P, f], dt, tag="vf")
                nc.vector.tensor_tensor(
                    out=vf[:], in0=ij[:, :f], in1=ij[:, f:], op=mult
                )
                vi = pool.tile([P, f], mybir.dt.int32, tag="vi")
                nc.vector.tensor_copy(out=vi[:], in_=vf[:])
                val = pool.tile([P, 2 * f], mybir.dt.int32, tag="val")
                # lo = v & 15 ; hi = v >> 4
                nc.vector.tensor_scalar(
                    out=val[:, :f], in0=vi[:], scalar1=15, scalar2=None,
                    op0=mybir.AluOpType.bitwise_and,
                )
                nc.vector.tensor_scalar(
                    out=val[:, f:], in0=vi[:], scalar1=4, scalar2=None,
                    op0=mybir.AluOpType.logical_shift_right,
                )
                out = pool.tile([P, 2 * fk], dt, tag="out")
                out3 = out[:].rearrange("p (s k) -> p s k", k=K)
                iota_b2 = iota_i[:, None, :].broadcast_to([P, f, K])
                # one-hot: out[p, s, k] = (val[p, s] == k), int compare, f32 out
                nc.vector.tensor_tensor(
                    out=out3[:, :f],
                    in0=val[:, :f, None].broadcast_to([P, f, K]),
                    in1=iota_b2,
                    op=mybir.AluOpType.is_equal,
                )
                eq_hi_eng.tensor_tensor(
                    out=out3[:, f:],
                    in0=val[:, f:, None].broadcast_to([P, f, K]),
                    in1=iota_b2,
                    op=mybir.AluOpType.is_equal,
                )
                st_eng = nc.scalar if variant == "dualq" else nc.sync
                st_eng.dma_start(out=lo_t[t], in_=out[:, :fk])
                st_eng.dma_start(out=hi_t[t], in_=out[:, fk:])
    nc.finalize()
    return nc


def _build_nc_bigload(nc, mybir, n_rows, f, repeats, a_in, b_in, iota_in,
                      lo_out, hi_out, bf=256):
    """Big DMA granularity (bf rows/partition), sub-tile compute (f rows)."""
    from concourse.tile import TileContext

    fk = f * K
    bfk = bf * K
    ns = bf // f  # compute sub-tiles per big tile
    assert n_rows % (P * bf) == 0
    TB = n_rows // (P * bf)
    dt = mybir.dt.float32
    mult = mybir.AluOpType.mult

    # DRAM views: big tile bt, partition p covers rows bt*P*bf + p*bf + [0, bf)
    a_t = a_in.rearrange("(t p s j) k -> t p (s j k)", p=P, s=ns, j=f)
    b_t = b_in.rearrange("(t p s j) k -> t p (s j k)", p=P, s=ns, j=f)
    lo_t = lo_out.rearrange("(t p s j) k -> t s p (j k)", p=P, s=ns, j=f)
    hi_t = hi_out.rearrange("(t p s j) k -> t s p (j k)", p=P, s=ns, j=f)

    with TileContext(nc) as tc:
        with (
            tc.tile_pool(name="const", bufs=1) as cpool,
            tc.tile_pool(name="big", bufs=2) as bpool,
            tc.tile_pool(name="work", bufs=3) as pool,
        ):
            iota = cpool.tile([P, K], dt)
            nc.sync.dma_start(out=iota[:], in_=iota_in)
            iota_i = cpool.tile([P, K], mybir.dt.int32)
            nc.gpsimd.iota(iota_i[:], pattern=[[1, K]], base=0, channel_multiplier=0)
            for t in [ti for _ in range(repeats) for ti in range(TB)]:
                ab = bpool.tile([P, 2 * bfk], dt, tag="abbig")
                nc.sync.dma_start(out=ab[:, :bfk], in_=a_t[t])
                nc.sync.dma_start(out=ab[:, bfk:], in_=b_t[t])
                # [P, 2(a|b), ns, f, K] view of the pair of big halves
                ab5 = ab[:].rearrange("p (h s j k) -> p h s j k", h=2, s=ns, k=K)
                for s in range(ns):
                    sub = ab5[:, :, s]  # [P, 2, f, K]
                    iota_b = iota[:, None, None, :].broadcast_to([P, 2, f, K])
                    nc.vector.tensor_tensor(out=sub, in0=sub, in1=iota_b, op=mult)
                    ij = pool.tile([P, 2 * f], dt, tag="ij")
                    nc.vector.tensor_reduce(
                        out=ij[:].rearrange("p (h j) -> p h j", h=2), in_=sub,
                        axis=mybir.AxisListType.X, op=mybir.AluOpType.add,
                    )
                    vf = pool.tile([P, f], dt, tag="vf")
                    nc.vector.tensor_tensor(
                        out=vf[:], in0=ij[:, :f], in1=ij[:, f:], op=mult
                    )
                    vi = pool.tile([P, f], mybir.dt.int32, tag="vi")
                    nc.vector.tensor_copy(out=vi[:], in_=vf[:])
                    val = pool.tile([P, 2 * f], mybir.dt.int32, tag="val")
                    nc.vector.tensor_scalar(
                        out=val[:, :f], in0=vi[:], scalar1=15, scalar2=None,
                        op0=mybir.AluOpType.bitwise_and,
                    )
                    nc.vector.tensor_scalar(
                        out=val[:, f:], in0=vi[:], scalar1=4, scalar2=None,
                        op0=mybir.AluOpType.logical_shift_right,
                    )
                    out = pool.tile([P, 2 * fk], dt, tag="out")
                    out3 = out[:].rearrange("p (q k) -> p q k", k=K)
                    iota_b2 = iota_i[:, None, :].broadcast_to([P, f, K])
                    nc.vector.tensor_tensor(
                        out=out3[:, :f],
                        in0=val[:, :f, None].broadcast_to([P, f, K]),
                        in1=iota_b2, op=mybir.AluOpType.is_equal,
                    )
                    nc.vector.tensor_tensor(
                        out=out3[:, f:],
                        in0=val[:, f:, None].broadcast_to([P, f, K]),
                        in1=iota_b2, op=mybir.AluOpType.is_equal,
                    )
                    nc.sync.dma_start(out=lo_t[t, s], in_=out[:, :fk])
                    nc.sync.dma_start(out=hi_t[t, s], in_=out[:, fk:])
    nc.finalize()
    return nc


def _get_nc(n_rows, **kw):
    key = (n_rows, tuple(sorted(kw.items())))
    if key not in _CACHE:
        _CACHE[key] = _build_nc(n_rows, **kw)
    return _CACHE[key]


def _iota_input():
    return np.tile(np.arange(K, dtype=np.float32), (P, 1))


def _expected_tables():
    W1 = np.zeros((32, 256), dtype=np.float32)
    W2_lo = np.zeros((256, 16), dtype=np.float32)
    W2_hi = np.zeros((256, 16), dtype=np.float32)
    for a in range(16):
        for b in range(16):
            idx = a * 16 + b
            W1[a, idx] = 1.0
            W1[16 + b, idx] = 1.0
            p = a * b
            W2_lo[idx, p & 15] = 1.0
            W2_hi[idx, (p >> 4) & 15] = 1.0
    return W1, W2_lo, W2_hi


def _numpy_fallback(a, b, W1, W2_lo, W2_hi):
    # Faithful recomputation of the reference for unexpected table contents.
    combined = np.concatenate([a, b], axis=-1).astype(np.float32)
    logits = (combined @ W1) * np.float32(100.0)
    logits -= logits.max(axis=-1, keepdims=True)
    e = np.exp(logits, dtype=np.float32)
    address = e / e.sum(axis=-1, keepdims=True, dtype=np.float32)
    return address @ W2_lo, address @ W2_hi


def kernel(a, b, W1, W2_lo, W2_hi):
    from concourse.bass_utils import run_bass_kernel_spmd

    a = np.ascontiguousarray(np.asarray(a, dtype=np.float32))
    b = np.ascontiguousarray(np.asarray(b, dtype=np.float32))

    W1e, W2loe, W2hie = _expected_tables()
    if not (
        np.array_equal(np.asarray(W1, np.float32), W1e)
        and np.array_equal(np.asarray(W2_lo, np.float32), W2loe)
        and np.array_equal(np.asarray(W2_hi, np.float32), W2hie)
    ):
        return _numpy_fallback(a, b, np.asarray(W1), np.asarray(W2_lo), np.asarray(W2_hi))

    n = a.shape[0]
    assert n == N_TOTAL and n % N_CORES == 0, a.shape
    n_core = n // N_CORES

    nc = _get_nc(n_core)
    iota = _iota_input()
    in_maps = [
        {
            "a_in": a[c * n_core : (c + 1) * n_core],
            "b_in": b[c * n_core : (c + 1) * n_core],
            "iota_in": iota,
        }
        for c in range(N_CORES)
    ]
    res = run_bass_kernel_spmd(nc, in_maps, core_ids=list(range(N_CORES)))
    lo = np.concatenate([r["lo_out"] for r in res.results], axis=0)
    hi = np.concatenate([r["hi_out"] for r in res.results], axis=0)
    return lo, hi
